# revision 10
# baseline (speedup 1.0000x reference)
"""CoAttention Trainium2 kernel (bf16 I/O, fused epilogue).

Computes A[b,i,j] = u[b,i,:]@w1 + v[b,j,:]@w2 + sum_d u[b,i,d]*w3[d]*v[b,j,d]
for u, v: [16, 2048, 256] f32, w1/w2/w3: [256] f32 -> A: [16, 2048, 2048] f32.

Sharding: batch dim (16) split across 8 NeuronCores (2 batches/core, data
parallel); w1/w2/w3 replicated.

Memory-regime strategy: the kernel is HBM-bound (output is 256 MiB), so all
device I/O is bf16 (rel-err gate is 2e-2; bf16 end-to-end lands ~3e-3):
  - host pre-transposes u,v to [D, S] layout and casts to bf16 (removes all
    PE transposes and halves input DMA)
  - output tensor is bf16 (halves the dominant store traffic), host upcasts
Device per batch:
  - vw3T[d,j] = w3[d]*vT[d,j] on ACT (per-partition scale)
  - w2vb[p,j] = sum_d w2[d] vT[d,j] via PE (w2 replicated stationary)
  - w1u[i] = sum_d uT[d,i] w1[d] via tiny N=1 matmuls (uT chunk stationary)
  - per 128-row i-block: psum[i,j] += uT_chunk^T @ vw3T_chunk (bf16, 1 cyc/row)
    epilogue: ONE fused op per tile out = (psum + w1u[i]) + w2vb[:,j],
    alternating DVE/Pool engines; 512 KiB bf16 store per i-block.
"""

import numpy as np
from contextlib import ExitStack

B, S, D = 16, 2048, 256
N_CORES = 8
BPC = B // N_CORES  # batches per core
P = 128
NB = S // P    # 16 i blocks
NCH = D // P   # 2 contraction chunks
FQ = 512       # matmul psum slice (one bank)
NQ = S // FQ   # 4 j quarters

_CACHE = {}


def _build(level=40):
    # level: bisect scaffolding. 1=DMA only, 2=+main matmuls, 3=+w2vb/w1u
    # builds, 40=full (fused epilogue)
    import concourse.bacc as bacc
    import concourse.mybir as mybir
    import concourse.tile as tile

    dt = mybir.dt
    f32 = dt.float32
    bf16 = dt.bfloat16
    ADD = mybir.AluOpType.add
    COPY = mybir.ActivationFunctionType.Copy

    nc = bacc.Bacc("TRN2", debug=False, num_devices=N_CORES)
    ut_d = nc.dram_tensor("ut", [BPC, D, S], bf16, kind="ExternalInput").ap()
    vt_d = nc.dram_tensor("vt", [BPC, D, S], bf16, kind="ExternalInput").ap()
    w1t_d = nc.dram_tensor("w1t", [P, NCH], bf16, kind="ExternalInput").ap()
    w2t_d = nc.dram_tensor("w2t", [P, NCH], f32, kind="ExternalInput").ap()
    w3t_d = nc.dram_tensor("w3t", [P, NCH], f32, kind="ExternalInput").ap()
    out_d = nc.dram_tensor("out", [BPC, S, S], bf16, kind="ExternalOutput").ap()

    with tile.TileContext(nc) as tc, ExitStack() as ctx:
        const = ctx.enter_context(tc.tile_pool(name="const", bufs=1))
        inp = ctx.enter_context(tc.tile_pool(name="inp", bufs=2))
        vw_pool = ctx.enter_context(tc.tile_pool(name="vw", bufs=2))
        work = ctx.enter_context(tc.tile_pool(name="work", bufs=2))
        outp = ctx.enter_context(tc.tile_pool(name="outp", bufs=4))
        psp = ctx.enter_context(tc.tile_pool(name="psp", bufs=2, space="PSUM"))

        # ---- constants ----
        w1t = const.tile([P, NCH], bf16, tag="w1t")
        nc.scalar.dma_start(out=w1t[:], in_=w1t_d)
        w3t = const.tile([P, NCH], f32, tag="w3t")
        nc.scalar.dma_start(out=w3t[:], in_=w3t_d)
        w2tc = const.tile([P, NCH], f32, tag="w2tc")
        nc.scalar.dma_start(out=w2tc[:], in_=w2t_d)
        ones = const.tile([P, P], bf16, tag="ones")
        nc.vector.memset(ones[:], 1.0)

        # w2t[d, ch, p] = w2[ch*128+d] (stationary operand for the w2v
        # broadcast: psum[p, j] += sum_d w2t[d,p] * vT[d,j])
        w2t = const.tile([P, NCH, P], bf16, tag="w2t")
        for ch in range(NCH):
            nc.vector.tensor_scalar(
                w2t[:, ch, :], ones[:], w2tc[:, ch:ch + 1], None,
                mybir.AluOpType.mult,
            )

        for bi in range(BPC):
            # whole-batch transposed input loads (1 MiB bf16 each, ACT ring)
            vt_sb = inp.tile([P, NCH, S], bf16, tag="vt")
            nc.scalar.dma_start(
                out=vt_sb[:], in_=vt_d[bi].rearrange("(ch p) s -> p ch s", p=P)
            )
            ut_sb = inp.tile([P, NCH, S], bf16, tag="ut")
            nc.scalar.dma_start(
                out=ut_sb[:], in_=ut_d[bi].rearrange("(ch p) s -> p ch s", p=P)
            )

            # vw3T[d, j] = w3[d] * vT[d, j]  (GpSimd, per-partition scale;
            # SBUF-only — GPSIMD cannot access PSUM on this target)
            vw3 = vw_pool.tile([P, NCH, S], bf16, tag="vw3")
            for ch in range(NCH):
                nc.gpsimd.tensor_scalar(
                    vw3[:, ch, :], vt_sb[:, ch, :], w3t[:, ch:ch + 1], None,
                    mybir.AluOpType.mult,
                )

            if level >= 3:
                # w2vb[p, j] = w2v[j] for all p (PE broadcast matmul)
                ps_w = psp.tile([P, S], f32, tag="ps", name=f"ps_w2v_{bi}")
                for q in range(NQ):
                    qs = slice(q * FQ, (q + 1) * FQ)
                    for ch in range(NCH):
                        nc.tensor.matmul(
                            ps_w[:, qs], lhsT=w2t[:, ch, :],
                            rhs=vt_sb[:, ch, qs],
                            start=(ch == 0), stop=(ch == NCH - 1),
                        )
                w2vb = work.tile([P, S], bf16, tag="w2vb")
                nc.scalar.activation(out=w2vb[:], in_=ps_w[:], func=COPY)

                # w1u[i] = sum_d uT[d,i] w1[d]; one N=1 matmul per (ib, ch)
                ps_w1 = psp.tile([P, NB], f32, tag="ps", name=f"ps_w1u_{bi}")
                for ib in range(NB):
                    for ch in range(NCH):
                        nc.tensor.matmul(
                            ps_w1[:, ib:ib + 1],
                            lhsT=ut_sb[:, ch, ib * P:(ib + 1) * P],
                            rhs=w1t[:, ch:ch + 1],
                            start=(ch == 0), stop=(ch == NCH - 1),
                        )
                w1u = work.tile([P, NB], f32, tag="w1u")
                nc.vector.tensor_copy(w1u[:], ps_w1[:])

            for ib in range(NB):
                ps = psp.tile([P, S], f32, tag="ps", name=f"ps_{bi}_{ib}")
                if level >= 2:
                    # ch-outer: stationary uT chunk held across 4 psum slices
                    for ch in range(NCH):
                        for q in range(NQ):
                            qs = slice(q * FQ, (q + 1) * FQ)
                            nc.tensor.matmul(
                                ps[:, qs],
                                lhsT=ut_sb[:, ch, ib * P:(ib + 1) * P],
                                rhs=vw3[:, ch, qs],
                                start=(ch == 0), stop=(ch == NCH - 1),
                            )
                orow = outp.tile([P, S], bf16, tag="orow")
                if level >= 40:
                    # epilogue: out = (psum + w1u[i]) + w2vb. GPSIMD cannot
                    # read PSUM, so alternate between
                    #   odd ib:  DVE single fused op (psum read)
                    #   even ib: ACT psum+bias -> orow, GpSimd += w2vb in place
                    if ib % 2 == 1:
                        nc.vector.scalar_tensor_tensor(
                            out=orow[:], in0=ps[:], scalar=w1u[:, ib:ib + 1],
                            in1=w2vb[:], op0=ADD, op1=ADD,
                        )
                    else:
                        nc.scalar.activation(
                            out=orow[:], in_=ps[:],
                            func=mybir.ActivationFunctionType.Identity,
                            bias=w1u[:, ib:ib + 1], scale=1.0,
                        )
                        nc.gpsimd.tensor_tensor(
                            out=orow[:], in0=orow[:], in1=w2vb[:], op=ADD
                        )
                elif level >= 2:
                    nc.vector.tensor_copy(orow[:], ps[:])
                else:
                    nc.vector.memset(orow[:], float(ib))
                nc.sync.dma_start(
                    out=out_d[bi, ib * P:(ib + 1) * P, :], in_=orow[:]
                )

    nc.compile()
    return nc


def _get_nc():
    if "nc" not in _CACHE:
        _CACHE["nc"] = _build()
    return _CACHE["nc"]


def kernel(u, v, w1, w2, w3, _trace=False, _trace_cores=None, _results_out=None):
    import ml_dtypes
    from concourse.bass_utils import run_bass_kernel_spmd

    bf16 = ml_dtypes.bfloat16
    nc = _get_nc()

    # host-side layout prep: cast to bf16, transpose to [D, S]
    ut = np.ascontiguousarray(
        np.asarray(u, dtype=np.float32).astype(bf16).transpose(0, 2, 1)
    )
    vt = np.ascontiguousarray(
        np.asarray(v, dtype=np.float32).astype(bf16).transpose(0, 2, 1)
    )
    w1t = np.ascontiguousarray(
        np.asarray(w1, dtype=np.float32).reshape(NCH, P).T
    ).astype(bf16)
    w3t = np.ascontiguousarray(
        np.asarray(w3, dtype=np.float32).reshape(NCH, P).T
    ).astype(np.float32)
    w2t = np.ascontiguousarray(
        np.asarray(w2, dtype=np.float32).reshape(NCH, P).T
    ).astype(np.float32)

    in_maps = [
        {
            "ut": np.ascontiguousarray(ut[c * BPC:(c + 1) * BPC]),
            "vt": np.ascontiguousarray(vt[c * BPC:(c + 1) * BPC]),
            "w1t": w1t,
            "w2t": w2t,
            "w3t": w3t,
        }
        for c in range(N_CORES)
    ]
    kw = {}
    if _trace:
        kw["trace"] = True
        if _trace_cores is not None:
            kw["trace_cores"] = _trace_cores
    res = run_bass_kernel_spmd(nc, in_maps, core_ids=list(range(N_CORES)), **kw)
    if _results_out is not None:
        _results_out.append(res)
    out = np.concatenate(
        [np.asarray(res.results[c]["out"]) for c in range(N_CORES)], axis=0
    )
    return out.astype(np.float32)


# revision 12
# speedup vs baseline: 2.0632x; 2.0632x over previous
"""CoAttention Trainium2 kernel (bf16 I/O, fused epilogue).

Computes A[b,i,j] = u[b,i,:]@w1 + v[b,j,:]@w2 + sum_d u[b,i,d]*w3[d]*v[b,j,d]
for u, v: [16, 2048, 256] f32, w1/w2/w3: [256] f32 -> A: [16, 2048, 2048] f32.

Sharding: batch dim (16) split across 8 NeuronCores (2 batches/core, data
parallel); w1/w2/w3 replicated.

Memory-regime strategy: the kernel is HBM-bound (output is 256 MiB), so all
device I/O is bf16 (rel-err gate is 2e-2; bf16 end-to-end lands ~3e-3):
  - host pre-transposes u,v to [D, S] layout and casts to bf16 (removes all
    PE transposes and halves input DMA)
  - output tensor is bf16 (halves the dominant store traffic), host upcasts
Device per batch:
  - vw3T[d,j] = w3[d]*vT[d,j] on ACT (per-partition scale)
  - w2vb[p,j] = sum_d w2[d] vT[d,j] via PE (w2 replicated stationary)
  - w1u[i] = sum_d uT[d,i] w1[d] via tiny N=1 matmuls (uT chunk stationary)
  - per 128-row i-block: psum[i,j] += uT_chunk^T @ vw3T_chunk (bf16, 1 cyc/row)
    epilogue: ONE fused op per tile out = (psum + w1u[i]) + w2vb[:,j],
    alternating DVE/Pool engines; 512 KiB bf16 store per i-block.
"""

import numpy as np
from contextlib import ExitStack

B, S, D = 16, 2048, 256
N_CORES = 8
BPC = B // N_CORES  # batches per core
P = 128
NB = S // P    # 16 i blocks
NCH = D // P   # 2 contraction chunks
FQ = 512       # matmul psum slice (one bank)
NQ = S // FQ   # 4 j quarters

_CACHE = {}


def _build(level=40):
    # level: bisect scaffolding. 1=DMA only, 2=+main matmuls, 3=+w2vb/w1u
    # builds, 40=full (fused epilogue)
    import concourse.bacc as bacc
    import concourse.mybir as mybir
    import concourse.tile as tile

    dt = mybir.dt
    f32 = dt.float32
    bf16 = dt.bfloat16
    ADD = mybir.AluOpType.add
    COPY = mybir.ActivationFunctionType.Copy

    nc = bacc.Bacc("TRN2", debug=False, num_devices=N_CORES)
    ut_d = nc.dram_tensor("ut", [BPC, D, S], bf16, kind="ExternalInput").ap()
    vt_d = nc.dram_tensor("vt", [BPC, D, S], bf16, kind="ExternalInput").ap()
    w1t_d = nc.dram_tensor("w1t", [P, NCH], bf16, kind="ExternalInput").ap()
    w2t_d = nc.dram_tensor("w2t", [P, NCH], f32, kind="ExternalInput").ap()
    w3t_d = nc.dram_tensor("w3t", [P, NCH], f32, kind="ExternalInput").ap()
    out_d = nc.dram_tensor("out", [BPC, S, S], bf16, kind="ExternalOutput").ap()

    with tile.TileContext(nc) as tc, ExitStack() as ctx:
        const = ctx.enter_context(tc.tile_pool(name="const", bufs=1))
        inp = ctx.enter_context(tc.tile_pool(name="inp", bufs=2))
        vw_pool = ctx.enter_context(tc.tile_pool(name="vw", bufs=2))
        work = ctx.enter_context(tc.tile_pool(name="work", bufs=2))
        outp = ctx.enter_context(tc.tile_pool(name="outp", bufs=4))
        psp = ctx.enter_context(tc.tile_pool(name="psp", bufs=2, space="PSUM"))

        # ---- constants ----
        w1t = const.tile([P, NCH], bf16, tag="w1t")
        nc.scalar.dma_start(out=w1t[:], in_=w1t_d)
        w3t = const.tile([P, NCH], f32, tag="w3t")
        nc.scalar.dma_start(out=w3t[:], in_=w3t_d)
        w2tc = const.tile([P, NCH], f32, tag="w2tc")
        nc.scalar.dma_start(out=w2tc[:], in_=w2t_d)
        ones = const.tile([P, P], bf16, tag="ones")
        nc.vector.memset(ones[:], 1.0)

        # w2t[d, ch, p] = w2[ch*128+d] (stationary operand for the w2v
        # broadcast: psum[p, j] += sum_d w2t[d,p] * vT[d,j])
        w2t = const.tile([P, NCH, P], bf16, tag="w2t")
        for ch in range(NCH):
            nc.vector.tensor_scalar(
                w2t[:, ch, :], ones[:], w2tc[:, ch:ch + 1], None,
                mybir.AluOpType.mult,
            )

        for bi in range(BPC):
            # whole-batch transposed input loads (1 MiB bf16 each, ACT ring)
            vt_sb = inp.tile([P, NCH, S], bf16, tag="vt")
            nc.scalar.dma_start(
                out=vt_sb[:], in_=vt_d[bi].rearrange("(ch p) s -> p ch s", p=P)
            )
            ut_sb = inp.tile([P, NCH, S], bf16, tag="ut")
            nc.scalar.dma_start(
                out=ut_sb[:], in_=ut_d[bi].rearrange("(ch p) s -> p ch s", p=P)
            )

            # vw3T[d, j] = w3[d] * vT[d, j]  (DVE, per-partition scale;
            # all-SBUF bf16 so DVE can use its fast path)
            vw3 = vw_pool.tile([P, NCH, S], bf16, tag="vw3")
            for ch in range(NCH):
                nc.vector.tensor_scalar(
                    vw3[:, ch, :], vt_sb[:, ch, :], w3t[:, ch:ch + 1], None,
                    mybir.AluOpType.mult,
                )

            if level >= 3:
                # w2vb[p, j] = w2v[j] for all p (PE broadcast matmul)
                ps_w = psp.tile([P, S], f32, tag="ps", name=f"ps_w2v_{bi}")
                for q in range(NQ):
                    qs = slice(q * FQ, (q + 1) * FQ)
                    for ch in range(NCH):
                        nc.tensor.matmul(
                            ps_w[:, qs], lhsT=w2t[:, ch, :],
                            rhs=vt_sb[:, ch, qs],
                            start=(ch == 0), stop=(ch == NCH - 1),
                        )
                w2vb = work.tile([P, S], bf16, tag="w2vb")
                nc.scalar.activation(out=w2vb[:], in_=ps_w[:], func=COPY)

                # w1u[i] = sum_d uT[d,i] w1[d]; one N=1 matmul per (ib, ch)
                ps_w1 = psp.tile([P, NB], f32, tag="ps", name=f"ps_w1u_{bi}")
                for ib in range(NB):
                    for ch in range(NCH):
                        nc.tensor.matmul(
                            ps_w1[:, ib:ib + 1],
                            lhsT=ut_sb[:, ch, ib * P:(ib + 1) * P],
                            rhs=w1t[:, ch:ch + 1],
                            start=(ch == 0), stop=(ch == NCH - 1),
                        )
                w1u = work.tile([P, NB], f32, tag="w1u")
                nc.vector.tensor_copy(w1u[:], ps_w1[:])

            for ib in range(NB):
                ps = psp.tile([P, S], f32, tag="ps", name=f"ps_{bi}_{ib}")
                if level >= 2:
                    # ch-outer: stationary uT chunk held across 4 psum slices
                    for ch in range(NCH):
                        for q in range(NQ):
                            qs = slice(q * FQ, (q + 1) * FQ)
                            nc.tensor.matmul(
                                ps[:, qs],
                                lhsT=ut_sb[:, ch, ib * P:(ib + 1) * P],
                                rhs=vw3[:, ch, qs],
                                start=(ch == 0), stop=(ch == NCH - 1),
                            )
                orow = outp.tile([P, S], bf16, tag="orow")
                if level >= 40:
                    # epilogue: out = (psum + w1u[i]) + w2vb. GPSIMD cannot
                    # read PSUM, so split:
                    #   most ib: DVE single fused op (psum read, ~2.35us)
                    #   ib%3==1: ACT psum+bias -> orow, GpSimd += w2vb in
                    #   place (GpSimd is slow: ~4us/tile, so only 5/16)
                    if ib % 3 != 1:
                        nc.vector.scalar_tensor_tensor(
                            out=orow[:], in0=ps[:], scalar=w1u[:, ib:ib + 1],
                            in1=w2vb[:], op0=ADD, op1=ADD,
                        )
                    else:
                        nc.scalar.activation(
                            out=orow[:], in_=ps[:],
                            func=mybir.ActivationFunctionType.Identity,
                            bias=w1u[:, ib:ib + 1], scale=1.0,
                        )
                        nc.gpsimd.tensor_tensor(
                            out=orow[:], in0=orow[:], in1=w2vb[:], op=ADD
                        )
                elif level >= 2:
                    nc.vector.tensor_copy(orow[:], ps[:])
                else:
                    nc.vector.memset(orow[:], float(ib))
                nc.sync.dma_start(
                    out=out_d[bi, ib * P:(ib + 1) * P, :], in_=orow[:]
                )

    nc.compile()
    return nc


def _get_nc():
    if "nc" not in _CACHE:
        _CACHE["nc"] = _build()
    return _CACHE["nc"]


def kernel(u, v, w1, w2, w3, _trace=False, _trace_cores=None, _results_out=None):
    import ml_dtypes
    from concourse.bass_utils import run_bass_kernel_spmd

    bf16 = ml_dtypes.bfloat16
    nc = _get_nc()

    # host-side layout prep: cast to bf16, transpose to [D, S]
    ut = np.ascontiguousarray(
        np.asarray(u, dtype=np.float32).astype(bf16).transpose(0, 2, 1)
    )
    vt = np.ascontiguousarray(
        np.asarray(v, dtype=np.float32).astype(bf16).transpose(0, 2, 1)
    )
    w1t = np.ascontiguousarray(
        np.asarray(w1, dtype=np.float32).reshape(NCH, P).T
    ).astype(bf16)
    w3t = np.ascontiguousarray(
        np.asarray(w3, dtype=np.float32).reshape(NCH, P).T
    ).astype(np.float32)
    w2t = np.ascontiguousarray(
        np.asarray(w2, dtype=np.float32).reshape(NCH, P).T
    ).astype(np.float32)

    in_maps = [
        {
            "ut": np.ascontiguousarray(ut[c * BPC:(c + 1) * BPC]),
            "vt": np.ascontiguousarray(vt[c * BPC:(c + 1) * BPC]),
            "w1t": w1t,
            "w2t": w2t,
            "w3t": w3t,
        }
        for c in range(N_CORES)
    ]
    kw = {}
    if _trace:
        kw["trace"] = True
        if _trace_cores is not None:
            kw["trace_cores"] = _trace_cores
    res = run_bass_kernel_spmd(nc, in_maps, core_ids=list(range(N_CORES)), **kw)
    if _results_out is not None:
        _results_out.append(res)
    out = np.concatenate(
        [np.asarray(res.results[c]["out"]) for c in range(N_CORES)], axis=0
    )
    return out.astype(np.float32)


# revision 13
# speedup vs baseline: 2.3457x; 1.1369x over previous
"""CoAttention Trainium2 kernel (bf16 I/O, fused epilogue).

Computes A[b,i,j] = u[b,i,:]@w1 + v[b,j,:]@w2 + sum_d u[b,i,d]*w3[d]*v[b,j,d]
for u, v: [16, 2048, 256] f32, w1/w2/w3: [256] f32 -> A: [16, 2048, 2048] f32.

Sharding: batch dim (16) split across 8 NeuronCores (2 batches/core, data
parallel); w1/w2/w3 replicated.

Memory-regime strategy: the kernel is HBM-bound (output is 256 MiB), so all
device I/O is bf16 (rel-err gate is 2e-2; bf16 end-to-end lands ~3e-3):
  - host pre-transposes u,v to [D, S] layout and casts to bf16 (removes all
    PE transposes and halves input DMA)
  - output tensor is bf16 (halves the dominant store traffic), host upcasts
Device per batch:
  - vw3T[d,j] = w3[d]*vT[d,j] on DVE (per-partition scale, bf16 fast path)
  - w2vb[p,j] = sum_d w2[d] vT[d,j] via PE (w2 replicated stationary)
  - w1u[i] = sum_d uT[d,i] w1[d] via tiny N=1 matmuls (uT chunk stationary)
  - per 128-row i-block: psum[i,j] += uT_chunk^T @ vw3T_chunk (bf16 matmuls
    stream at ~216ns/512-row on the PE)
PSUM is organized as [128,1024] half-tiles with a 4-deep rotation so the
epilogue drain latency hides behind 3 half-tiles of PE work. Drain paths
(GPSIMD cannot read PSUM; ACT can only add per-partition bias):
  a) DVE scalar_tensor_tensor: orow = (psum + w1u[i]) + w2vb   (~1.3us)
  b) ACT bias (psum+w1u -> orow bf16), GpSimd orow += w2vb     (ACT 1.1 + GP 2.1)
  c) ACT bias (psum+w1u -> orow bf16), DVE orow += w2vb        (ACT 1.1 + DVE ~0.4)
mixed a:c:b = 2:2:1 to balance DVE/ACT/GP busy under the PE roofline.
One 512 KiB bf16 store per i-block on the sync ring.
"""

import numpy as np
from contextlib import ExitStack

B, S, D = 16, 2048, 256
N_CORES = 8
BPC = B // N_CORES  # batches per core
P = 128
NB = S // P    # 16 i blocks
NCH = D // P   # 2 contraction chunks
FQ = 512       # matmul psum slice (one bank)
HQ = 1024      # psum half-tile width
NH = S // HQ   # 2 halves per i-block

_CACHE = {}


def _build(level=40):
    import concourse.bacc as bacc
    import concourse.mybir as mybir
    import concourse.tile as tile

    dt = mybir.dt
    f32 = dt.float32
    bf16 = dt.bfloat16
    ADD = mybir.AluOpType.add
    MULT = mybir.AluOpType.mult
    IDENT = mybir.ActivationFunctionType.Identity
    COPY = mybir.ActivationFunctionType.Copy

    nc = bacc.Bacc("TRN2", debug=False, num_devices=N_CORES)
    ut_d = nc.dram_tensor("ut", [BPC, D, S], bf16, kind="ExternalInput").ap()
    vt_d = nc.dram_tensor("vt", [BPC, D, S], bf16, kind="ExternalInput").ap()
    w1t_d = nc.dram_tensor("w1t", [P, NCH], bf16, kind="ExternalInput").ap()
    w2t_d = nc.dram_tensor("w2t", [P, NCH], f32, kind="ExternalInput").ap()
    w3t_d = nc.dram_tensor("w3t", [P, NCH], f32, kind="ExternalInput").ap()
    out_d = nc.dram_tensor("out", [BPC, S, S], bf16, kind="ExternalOutput").ap()

    with tile.TileContext(nc) as tc, ExitStack() as ctx:
        const = ctx.enter_context(tc.tile_pool(name="const", bufs=1))
        inp = ctx.enter_context(tc.tile_pool(name="inp", bufs=2))
        vw_pool = ctx.enter_context(tc.tile_pool(name="vw", bufs=2))
        work = ctx.enter_context(tc.tile_pool(name="work", bufs=2))
        outp = ctx.enter_context(tc.tile_pool(name="outp", bufs=4))
        psp = ctx.enter_context(tc.tile_pool(name="psp", bufs=4, space="PSUM"))

        # batch-0 inputs first: the first PE work (w2vb) waits on vt0
        vt_sbs, ut_sbs = [], []
        for bi in range(BPC):
            vt_sb = inp.tile([P, NCH, S], bf16, tag="vt", name=f"vt{bi}")
            nc.scalar.dma_start(
                out=vt_sb[:], in_=vt_d[bi].rearrange("(ch p) s -> p ch s", p=P)
            )
            ut_sb = inp.tile([P, NCH, S], bf16, tag="ut", name=f"ut{bi}")
            nc.scalar.dma_start(
                out=ut_sb[:], in_=ut_d[bi].rearrange("(ch p) s -> p ch s", p=P)
            )
            vt_sbs.append(vt_sb)
            ut_sbs.append(ut_sb)

        # ---- constants (tiny; on the sync ring to stay off the input path)
        w1t = const.tile([P, NCH], bf16, tag="w1t")
        nc.sync.dma_start(out=w1t[:], in_=w1t_d)
        w3t = const.tile([P, NCH], f32, tag="w3t")
        nc.sync.dma_start(out=w3t[:], in_=w3t_d)
        w2tc = const.tile([P, NCH], f32, tag="w2tc")
        nc.sync.dma_start(out=w2tc[:], in_=w2t_d)
        ones = const.tile([P, P], bf16, tag="ones")
        nc.vector.memset(ones[:], 1.0)

        # w2t[d, ch, p] = w2[ch*128+d] (stationary operand for the w2v
        # broadcast: psum[p, j] += sum_d w2t[d,p] * vT[d,j])
        w2t = const.tile([P, NCH, P], bf16, tag="w2t")
        for ch in range(NCH):
            nc.vector.tensor_scalar(
                w2t[:, ch, :], ones[:], w2tc[:, ch:ch + 1], None, MULT,
            )

        for bi in range(BPC):
            vt_sb, ut_sb = vt_sbs[bi], ut_sbs[bi]

            # vw3T[d, j] = w3[d] * vT[d, j]  (DVE per-partition scale, bf16)
            vw3 = vw_pool.tile([P, NCH, S], bf16, tag="vw3")
            for ch in range(NCH):
                nc.vector.tensor_scalar(
                    vw3[:, ch, :], vt_sb[:, ch, :], w3t[:, ch:ch + 1], None,
                    MULT,
                )

            # w2vb[p, j] = w2v[j] for all p (PE broadcast matmul, 2 halves)
            w2vb = work.tile([P, S], bf16, tag="w2vb")
            for jh in range(NH):
                ps_w = psp.tile([P, HQ], f32, tag="ps", name=f"ps_w2v_{bi}_{jh}")
                for q in range(2):
                    qs_p = slice(q * FQ, (q + 1) * FQ)
                    qs_v = slice(jh * HQ + q * FQ, jh * HQ + (q + 1) * FQ)
                    for ch in range(NCH):
                        nc.tensor.matmul(
                            ps_w[:, qs_p], lhsT=w2t[:, ch, :],
                            rhs=vt_sb[:, ch, qs_v],
                            start=(ch == 0), stop=(ch == NCH - 1),
                        )
                nc.scalar.activation(
                    out=w2vb[:, jh * HQ:(jh + 1) * HQ], in_=ps_w[:], func=COPY
                )

            # w1u[i] = sum_d uT[d,i] w1[d]; one N=1 matmul per (ib, ch)
            ps_w1 = psp.tile([P, NB], f32, tag="ps", name=f"ps_w1u_{bi}")
            for ib in range(NB):
                for ch in range(NCH):
                    nc.tensor.matmul(
                        ps_w1[:, ib:ib + 1],
                        lhsT=ut_sb[:, ch, ib * P:(ib + 1) * P],
                        rhs=w1t[:, ch:ch + 1],
                        start=(ch == 0), stop=(ch == NCH - 1),
                    )
            w1u = work.tile([P, NB], f32, tag="w1u")
            nc.vector.tensor_copy(w1u[:], ps_w1[:])

            for ib in range(NB):
                orow = outp.tile([P, S], bf16, tag="orow")
                for jh in range(NH):
                    ps = psp.tile(
                        [P, HQ], f32, tag="ps", name=f"ps_{bi}_{ib}_{jh}"
                    )
                    # ch-outer: stationary uT chunk held across both slices
                    for ch in range(NCH):
                        for q in range(2):
                            qs_p = slice(q * FQ, (q + 1) * FQ)
                            qs_v = slice(
                                jh * HQ + q * FQ, jh * HQ + (q + 1) * FQ
                            )
                            nc.tensor.matmul(
                                ps[:, qs_p],
                                lhsT=ut_sb[:, ch, ib * P:(ib + 1) * P],
                                rhs=vw3[:, ch, qs_v],
                                start=(ch == 0), stop=(ch == NCH - 1),
                            )
                    js = slice(jh * HQ, (jh + 1) * HQ)
                    idx = ib * NH + jh
                    path = ("a", "c", "a", "c", "b")[idx % 5]
                    if path == "a":
                        nc.vector.scalar_tensor_tensor(
                            out=orow[:, js], in0=ps[:],
                            scalar=w1u[:, ib:ib + 1],
                            in1=w2vb[:, js], op0=ADD, op1=ADD,
                        )
                    else:
                        nc.scalar.activation(
                            out=orow[:, js], in_=ps[:], func=IDENT,
                            bias=w1u[:, ib:ib + 1], scale=1.0,
                        )
                        eng = nc.vector if path == "c" else nc.gpsimd
                        eng.tensor_tensor(
                            out=orow[:, js], in0=orow[:, js],
                            in1=w2vb[:, js], op=ADD,
                        )
                nc.sync.dma_start(
                    out=out_d[bi, ib * P:(ib + 1) * P, :], in_=orow[:]
                )

    nc.compile()
    return nc


def _get_nc():
    if "nc" not in _CACHE:
        _CACHE["nc"] = _build()
    return _CACHE["nc"]


def kernel(u, v, w1, w2, w3, _trace=False, _trace_cores=None, _results_out=None):
    import ml_dtypes
    from concourse.bass_utils import run_bass_kernel_spmd

    bf16 = ml_dtypes.bfloat16
    nc = _get_nc()

    # host-side layout prep: cast to bf16, transpose to [D, S]
    ut = np.ascontiguousarray(
        np.asarray(u, dtype=np.float32).astype(bf16).transpose(0, 2, 1)
    )
    vt = np.ascontiguousarray(
        np.asarray(v, dtype=np.float32).astype(bf16).transpose(0, 2, 1)
    )
    w1t = np.ascontiguousarray(
        np.asarray(w1, dtype=np.float32).reshape(NCH, P).T
    ).astype(bf16)
    w3t = np.ascontiguousarray(
        np.asarray(w3, dtype=np.float32).reshape(NCH, P).T
    ).astype(np.float32)
    w2t = np.ascontiguousarray(
        np.asarray(w2, dtype=np.float32).reshape(NCH, P).T
    ).astype(np.float32)

    in_maps = [
        {
            "ut": np.ascontiguousarray(ut[c * BPC:(c + 1) * BPC]),
            "vt": np.ascontiguousarray(vt[c * BPC:(c + 1) * BPC]),
            "w1t": w1t,
            "w2t": w2t,
            "w3t": w3t,
        }
        for c in range(N_CORES)
    ]
    kw = {}
    if _trace:
        kw["trace"] = True
        if _trace_cores is not None:
            kw["trace_cores"] = _trace_cores
    res = run_bass_kernel_spmd(nc, in_maps, core_ids=list(range(N_CORES)), **kw)
    if _results_out is not None:
        _results_out.append(res)
    out = np.concatenate(
        [np.asarray(res.results[c]["out"]) for c in range(N_CORES)], axis=0
    )
    return out.astype(np.float32)


# revision 14
# speedup vs baseline: 2.4551x; 1.0466x over previous
"""CoAttention Trainium2 kernel (bf16 I/O, fused epilogue).

Computes A[b,i,j] = u[b,i,:]@w1 + v[b,j,:]@w2 + sum_d u[b,i,d]*w3[d]*v[b,j,d]
for u, v: [16, 2048, 256] f32, w1/w2/w3: [256] f32 -> A: [16, 2048, 2048] f32.

Sharding: batch dim (16) split across 8 NeuronCores (2 batches/core, data
parallel); w1/w2/w3 replicated.

Memory-regime strategy: the kernel is HBM-bound (output is 256 MiB), so all
device I/O is bf16 (rel-err gate is 2e-2; bf16 end-to-end lands ~3e-3):
  - host pre-transposes u,v to [D, S] layout and casts to bf16 (removes all
    PE transposes and halves input DMA)
  - output tensor is bf16 (halves the dominant store traffic), host upcasts
Device per batch:
  - vw3T[d,j] = w3[d]*vT[d,j] on DVE (per-partition scale, bf16 fast path)
  - w2vb[p,j] = sum_d w2[d] vT[d,j] via PE (w2 replicated stationary)
  - w1u[i] = sum_d uT[d,i] w1[d] via tiny N=1 matmuls (uT chunk stationary)
  - per 128-row i-block: psum[i,j] += uT_chunk^T @ vw3T_chunk (bf16 matmuls
    stream at ~216ns/512-row on the PE)
PSUM is organized as [128,1024] half-tiles with a 4-deep rotation so the
epilogue drain latency hides behind 3 half-tiles of PE work. Drain paths
(GPSIMD cannot read PSUM; ACT can only add per-partition bias):
  a) DVE scalar_tensor_tensor: orow = (psum + w1u[i]) + w2vb   (~1.3us)
  b) ACT bias (psum+w1u -> orow bf16), GpSimd orow += w2vb     (ACT 1.1 + GP 2.1)
  c) ACT bias (psum+w1u -> orow bf16), DVE orow += w2vb        (ACT 1.1 + DVE ~0.4)
mixed a:c:b = 2:2:1 to balance DVE/ACT/GP busy under the PE roofline.
One 512 KiB bf16 store per i-block on the sync ring.
"""

import numpy as np
from contextlib import ExitStack

B, S, D = 16, 2048, 256
N_CORES = 8
BPC = B // N_CORES  # batches per core
P = 128
NB = S // P    # 16 i blocks
NCH = D // P   # 2 contraction chunks
FQ = 512       # matmul psum slice (one bank)
HQ = 1024      # psum half-tile width
NH = S // HQ   # 2 halves per i-block

_CACHE = {}


def _build(level=40):
    import concourse.bacc as bacc
    import concourse.mybir as mybir
    import concourse.tile as tile

    dt = mybir.dt
    f32 = dt.float32
    bf16 = dt.bfloat16
    ADD = mybir.AluOpType.add
    MULT = mybir.AluOpType.mult
    IDENT = mybir.ActivationFunctionType.Identity
    COPY = mybir.ActivationFunctionType.Copy

    nc = bacc.Bacc("TRN2", debug=False, num_devices=N_CORES)
    ut_d = nc.dram_tensor("ut", [BPC, D, S], bf16, kind="ExternalInput").ap()
    vt_d = nc.dram_tensor("vt", [BPC, D, S], bf16, kind="ExternalInput").ap()
    w1t_d = nc.dram_tensor("w1t", [P, NCH], bf16, kind="ExternalInput").ap()
    w2t_d = nc.dram_tensor("w2t", [P, NCH], f32, kind="ExternalInput").ap()
    w3t_d = nc.dram_tensor("w3t", [P, NCH], f32, kind="ExternalInput").ap()
    out_d = nc.dram_tensor("out", [BPC, S, S], bf16, kind="ExternalOutput").ap()

    with tile.TileContext(nc) as tc, ExitStack() as ctx:
        const = ctx.enter_context(tc.tile_pool(name="const", bufs=1))
        inp = ctx.enter_context(tc.tile_pool(name="inp", bufs=2))
        vw_pool = ctx.enter_context(tc.tile_pool(name="vw", bufs=2))
        work = ctx.enter_context(tc.tile_pool(name="work", bufs=2))
        outp = ctx.enter_context(tc.tile_pool(name="outp", bufs=4))
        psp = ctx.enter_context(tc.tile_pool(name="psp", bufs=4, space="PSUM"))

        # ---- constants first: 24-byte loads, must not queue behind the
        # 4.2 MB input transfers (w2t gates the first PE matmul)
        w1t = const.tile([P, NCH], bf16, tag="w1t")
        nc.sync.dma_start(out=w1t[:], in_=w1t_d)
        w3t = const.tile([P, NCH], f32, tag="w3t")
        nc.sync.dma_start(out=w3t[:], in_=w3t_d)
        w2tc = const.tile([P, NCH], f32, tag="w2tc")
        nc.sync.dma_start(out=w2tc[:], in_=w2t_d)
        ones = const.tile([P, P], bf16, tag="ones")
        nc.vector.memset(ones[:], 1.0)

        # batch-0 inputs next: the first PE work (w2vb) waits on vt0.
        # All loads ride the sync ring; the scalar ring's ACT_TABLE_LOAD
        # would delay dispatch by ~1.3us.
        vt_sbs, ut_sbs = [], []
        for bi in range(BPC):
            vt_sb = inp.tile([P, NCH, S], bf16, tag="vt", name=f"vt{bi}")
            nc.sync.dma_start(
                out=vt_sb[:], in_=vt_d[bi].rearrange("(ch p) s -> p ch s", p=P)
            )
            ut_sb = inp.tile([P, NCH, S], bf16, tag="ut", name=f"ut{bi}")
            nc.sync.dma_start(
                out=ut_sb[:], in_=ut_d[bi].rearrange("(ch p) s -> p ch s", p=P)
            )
            vt_sbs.append(vt_sb)
            ut_sbs.append(ut_sb)

        # w2t[d, ch, p] = w2[ch*128+d] (stationary operand for the w2v
        # broadcast: psum[p, j] += sum_d w2t[d,p] * vT[d,j])
        w2t = const.tile([P, NCH, P], bf16, tag="w2t")
        for ch in range(NCH):
            nc.vector.tensor_scalar(
                w2t[:, ch, :], ones[:], w2tc[:, ch:ch + 1], None, MULT,
            )

        for bi in range(BPC):
            vt_sb, ut_sb = vt_sbs[bi], ut_sbs[bi]

            # vw3T[d, j] = w3[d] * vT[d, j]  (DVE per-partition scale, bf16)
            vw3 = vw_pool.tile([P, NCH, S], bf16, tag="vw3")
            for ch in range(NCH):
                nc.vector.tensor_scalar(
                    vw3[:, ch, :], vt_sb[:, ch, :], w3t[:, ch:ch + 1], None,
                    MULT,
                )

            # w2vb[p, j] = w2v[j] for all p (PE broadcast matmul, 2 halves)
            w2vb = work.tile([P, S], bf16, tag="w2vb")
            for jh in range(NH):
                ps_w = psp.tile([P, HQ], f32, tag="ps", name=f"ps_w2v_{bi}_{jh}")
                for q in range(2):
                    qs_p = slice(q * FQ, (q + 1) * FQ)
                    qs_v = slice(jh * HQ + q * FQ, jh * HQ + (q + 1) * FQ)
                    for ch in range(NCH):
                        nc.tensor.matmul(
                            ps_w[:, qs_p], lhsT=w2t[:, ch, :],
                            rhs=vt_sb[:, ch, qs_v],
                            start=(ch == 0), stop=(ch == NCH - 1),
                        )
                nc.scalar.activation(
                    out=w2vb[:, jh * HQ:(jh + 1) * HQ], in_=ps_w[:], func=COPY
                )

            # w1u[i] = sum_d uT[d,i] w1[d]; one N=1 matmul per (ib, ch)
            ps_w1 = psp.tile([P, NB], f32, tag="ps", name=f"ps_w1u_{bi}")
            for ib in range(NB):
                for ch in range(NCH):
                    nc.tensor.matmul(
                        ps_w1[:, ib:ib + 1],
                        lhsT=ut_sb[:, ch, ib * P:(ib + 1) * P],
                        rhs=w1t[:, ch:ch + 1],
                        start=(ch == 0), stop=(ch == NCH - 1),
                    )
            w1u = work.tile([P, NB], f32, tag="w1u")
            nc.vector.tensor_copy(w1u[:], ps_w1[:])

            for ib in range(NB):
                orow = outp.tile([P, S], bf16, tag="orow")
                for jh in range(NH):
                    ps = psp.tile(
                        [P, HQ], f32, tag="ps", name=f"ps_{bi}_{ib}_{jh}"
                    )
                    # ch-outer: stationary uT chunk held across both slices
                    for ch in range(NCH):
                        for q in range(2):
                            qs_p = slice(q * FQ, (q + 1) * FQ)
                            qs_v = slice(
                                jh * HQ + q * FQ, jh * HQ + (q + 1) * FQ
                            )
                            nc.tensor.matmul(
                                ps[:, qs_p],
                                lhsT=ut_sb[:, ch, ib * P:(ib + 1) * P],
                                rhs=vw3[:, ch, qs_v],
                                start=(ch == 0), stop=(ch == NCH - 1),
                            )
                    js = slice(jh * HQ, (jh + 1) * HQ)
                    idx = ib * NH + jh
                    path = ("a", "c", "a", "c", "b")[idx % 5]
                    if path == "a":
                        nc.vector.scalar_tensor_tensor(
                            out=orow[:, js], in0=ps[:],
                            scalar=w1u[:, ib:ib + 1],
                            in1=w2vb[:, js], op0=ADD, op1=ADD,
                        )
                    else:
                        nc.scalar.activation(
                            out=orow[:, js], in_=ps[:], func=IDENT,
                            bias=w1u[:, ib:ib + 1], scale=1.0,
                        )
                        eng = nc.vector if path == "c" else nc.gpsimd
                        eng.tensor_tensor(
                            out=orow[:, js], in0=orow[:, js],
                            in1=w2vb[:, js], op=ADD,
                        )
                nc.sync.dma_start(
                    out=out_d[bi, ib * P:(ib + 1) * P, :], in_=orow[:]
                )

    nc.compile()
    return nc


def _get_nc():
    if "nc" not in _CACHE:
        _CACHE["nc"] = _build()
    return _CACHE["nc"]


def kernel(u, v, w1, w2, w3, _trace=False, _trace_cores=None, _results_out=None):
    import ml_dtypes
    from concourse.bass_utils import run_bass_kernel_spmd

    bf16 = ml_dtypes.bfloat16
    nc = _get_nc()

    # host-side layout prep: cast to bf16, transpose to [D, S]
    ut = np.ascontiguousarray(
        np.asarray(u, dtype=np.float32).astype(bf16).transpose(0, 2, 1)
    )
    vt = np.ascontiguousarray(
        np.asarray(v, dtype=np.float32).astype(bf16).transpose(0, 2, 1)
    )
    w1t = np.ascontiguousarray(
        np.asarray(w1, dtype=np.float32).reshape(NCH, P).T
    ).astype(bf16)
    w3t = np.ascontiguousarray(
        np.asarray(w3, dtype=np.float32).reshape(NCH, P).T
    ).astype(np.float32)
    w2t = np.ascontiguousarray(
        np.asarray(w2, dtype=np.float32).reshape(NCH, P).T
    ).astype(np.float32)

    in_maps = [
        {
            "ut": np.ascontiguousarray(ut[c * BPC:(c + 1) * BPC]),
            "vt": np.ascontiguousarray(vt[c * BPC:(c + 1) * BPC]),
            "w1t": w1t,
            "w2t": w2t,
            "w3t": w3t,
        }
        for c in range(N_CORES)
    ]
    kw = {}
    if _trace:
        kw["trace"] = True
        if _trace_cores is not None:
            kw["trace_cores"] = _trace_cores
    res = run_bass_kernel_spmd(nc, in_maps, core_ids=list(range(N_CORES)), **kw)
    if _results_out is not None:
        _results_out.append(res)
    out = np.concatenate(
        [np.asarray(res.results[c]["out"]) for c in range(N_CORES)], axis=0
    )
    return out.astype(np.float32)
